# revision 11
# baseline (speedup 1.0000x reference)
"""Category-specific linear (MoE-style routed batched matmul) on 8 trn2 cores.

out[b, s, h] = sum_i x[b, s, i] * W[cat_ids[b], i, h] + bias[cat_ids[b], h]

Shapes (hardcoded): x (32, 512, 1024) f32, cat_ids (32,) int, W (16, 1024, 4096)
f32, b (16, 4096) f32 -> out (32, 512, 4096) f32.

Strategy: data-parallel over batch, 4 batches per core, with host-side routing
that always packs one same-category PAIR of batches plus two singles per core
(slot capacities [2, 1, 1] batches). With 32 batches over 16 categories there
are always >= (32 - 16)/2 = 8 disjoint same-category pairs, so this packing is
feasible for ANY cat_ids. Each core then loads only 3 weight matrices (24 MB
fp16), keeping DMA (~50 MB/core with fp16 output) under the PE floor
(1024 N=512 matmuls ~= 221 us back-to-back at 78.6 TF/s bf16).

v4 layout (driven by trace analysis):
  - All inputs are host-relaid so every DMA reads 8-32 KB contiguous per
    partition (2 KB-line chunk DMAs measured only ~125-150 GB/s/queue and
    starved the PE during the opening): xt as [P, KT, S] per batch, main W
    halves as [half, P, KT, 2048] (one 4 MB DMA each), plus a duplicated
    opening copy of slot-A half 0 as [nh, P, KT, 1024] (four 1 MB DMAs).
  - Opening (slot A, half 0) runs nh-chunk-major so the PE starts ~14 us in,
    right as the first 2 MB land; a 16-matmul fp16 warmup keeps the PE busy
    from ~8 us so the HAM clock gate flips once and stays at 2.4 GHz.
  - The opening bias broadcast rides the sync HWDGE queue AFTER the first W
    chunks (the SWDGE broadcast used to hog ~235 GB/s of fabric for 10 us
    exactly when W was critical).
  - psum as [128, 1024] 2-bank tiles, two accumulation groups per tile (zero
    regions are per-bank so the groups stay independent).
  - Output is stored fp16 (halves store traffic; host upcasts; ~5e-4 total
    relative error against a 2e-2 budget).
"""

import numpy as np

import concourse.bacc as bacc
import concourse.mybir as mybir
import concourse.bass as bass
import concourse.tile as tile
from concourse.bass_utils import run_bass_kernel_spmd

N_CORES = 8
B, S, K, H = 32, 512, 1024, 4096
BPC = B // N_CORES          # batches per core
P = 128                     # partitions
KT = K // P                 # k tiles (8)
MT = S // P                 # sample tiles per batch (4)
NHALF = 2                   # n halves per weight matrix
NH = H // NHALF             # cols per half (2048)
NH2 = NH // 2               # opening chunk width (1024)
SLOT_BATCHES = (2, 1, 1)    # batches per weight slot
NSLOT = len(SLOT_BATCHES)

_COMPILED = None


def _build():
    nc = bacc.Bacc("TRN2", target_bir_lowering=False, debug=False)
    f32 = mybir.dt.float32
    f16 = mybir.dt.float16

    # Host-relaid layouts: per-partition-contiguous lines (8-32 KB).
    xt_ap = nc.dram_tensor("xt", [BPC, P, KT, S], f16, kind="ExternalInput").ap()
    w_ap = nc.dram_tensor(
        "w", [NSLOT, NHALF, P, KT, NH], f16, kind="ExternalInput"
    ).ap()
    wo_ap = nc.dram_tensor("wo", [2, P, KT, NH2], f16, kind="ExternalInput").ap()
    bias_ap = nc.dram_tensor("bias", [NSLOT, H], f32, kind="ExternalInput").ap()
    out_ap = nc.dram_tensor("out", [BPC, S, H], f16, kind="ExternalOutput").ap()

    def bias_bcast_ap(s, half):
        src = bias_ap[s, half * NH : (half + 1) * NH]
        return bass.AP(tensor=src.tensor, offset=src.offset,
                       ap=[[0, P]] + list(src.ap))

    with tile.TileContext(nc) as tc:
        with (
            tc.tile_pool(name="warm_pool", bufs=1) as warm_pool,
            tc.tile_pool(name="xt_pool", bufs=4) as xt_pool,
            tc.tile_pool(name="wo_pool", bufs=4) as wo_pool,     # opening W chunks
            tc.tile_pool(name="wm_pool", bufs=2) as wm_pool,     # main W halves
            tc.tile_pool(name="bias_pool", bufs=2) as bias_pool,
            tc.tile_pool(name="outo_pool", bufs=4) as outo_pool,
            tc.tile_pool(name="outm_pool", bufs=4) as outm_pool,
            tc.tile_pool(name="ps_pool", bufs=4, space="PSUM") as ps_pool,
        ):
            # ---- warmup: flip the HAM clock gate while the first DMAs land.
            warm_x = warm_pool.tile([P, P], f16, name="warm_x", tag="warmx")
            warm_w = warm_pool.tile([P, 512], f16, tag="warmw", name="warm_w")
            nc.vector.memset(warm_x[:], 0.0)
            nc.vector.memset(warm_w[:], 0.0)
            warm_ps = ps_pool.tile([P, NH2], f32, tag="ps", name="warm_ps")
            for _ in range(16):
                nc.tensor.matmul(
                    warm_ps[:, 0:512], warm_x[:], warm_w[:], start=True, stop=True,
                    skip_group_check=True,
                )
            warm_out = warm_pool.tile([P, 4], f32, name="warm_out", tag="warmo")
            nc.vector.tensor_copy(warm_out[:], warm_ps[:, 0:4])

            # ---- opening DMAs. Sync-queue FIFO defines the arrival order:
            # xt_b0, W nh0 (2 x 1 MB), bias(A,h0) broadcast, xt_b1, W nh1.
            xt_ts = []
            for b in range(BPC):
                xt_ts.append(xt_pool.tile([P, KT, S], f16, name="xt_t", tag="xt"))
            nc.sync.dma_start(xt_ts[0][:], xt_ap[0])
            wo = [[], []]     # [nh][j] -> [P, 4, NH2] (kt in [4j, 4j+4))
            for j in range(2):
                wt = wo_pool.tile([P, 4, NH2], f16, tag="wo", name="wo0")
                nc.sync.dma_start(wt[:], wo_ap[0, :, 4 * j : 4 * (j + 1), :])
                wo[0].append(wt)
            bias_a0 = bias_pool.tile([P, NH], f32, name="bias_a0")
            nc.sync.dma_start(bias_a0[:], bias_bcast_ap(0, 0))
            nc.sync.dma_start(xt_ts[1][:], xt_ap[1])
            for j in range(2):
                wt = wo_pool.tile([P, 4, NH2], f16, tag="wo", name="wo1")
                nc.sync.dma_start(wt[:], wo_ap[1, :, 4 * j : 4 * (j + 1), :])
                wo[1].append(wt)
            # xt for slots B/C ride the (idle) scalar ring.
            nc.scalar.dma_start(xt_ts[2][:], xt_ap[2])
            nc.scalar.dma_start(xt_ts[3][:], xt_ap[3])

            def lhsT(b, kt, m):
                return xt_ts[b][:, kt, m * P : (m + 1) * P]

            # ---- opening compute: slot A half 0, nh-major, kt-outer so each
            # arriving chunk unlocks matmuls for all 4 m-tiles.
            for nh in range(2):
                for b in range(2):
                    ps = [
                        ps_pool.tile([P, NH2], f32, tag="ps", name="ps")
                        for _ in range(MT)
                    ]
                    for kt in range(KT):
                        w_src = wo[nh][kt // 4]
                        for m in range(MT):
                            lt = lhsT(b, kt, m)
                            for n2 in range(2):
                                nc.tensor.matmul(
                                    ps[m][:, n2 * 512 : (n2 + 1) * 512],
                                    lt,
                                    w_src[:, kt % 4, n2 * 512 : (n2 + 1) * 512],
                                    start=(kt == 0),
                                    stop=(kt == KT - 1),
                                )
                    for m in range(MT):
                        out_t = outo_pool.tile([P, NH2], f16, tag="outo")
                        nc.vector.tensor_add(
                            out_t[:], ps[m][:], bias_a0[:, nh * NH2 : (nh + 1) * NH2]
                        )
                        nc.scalar.dma_start(
                            out_ap[b, m * P : (m + 1) * P, nh * NH2 : (nh + 1) * NH2],
                            out_t[:],
                        )

            # ---- main phase: m-major, whole-half W tiles, double-buffered.
            slot_first_batch = (0, 2, 3)
            main_halves = [(0, 1), (1, 0), (1, 1), (2, 0), (2, 1)]
            for s, half in main_halves:
                nb = SLOT_BATCHES[s]
                bi0 = slot_first_batch[s]
                w_t = wm_pool.tile([P, KT, NH], f16, tag="wm", name="w_t")
                nc.sync.dma_start(w_t[:], w_ap[s, half])
                bias_t = bias_pool.tile([P, NH], f32, name="bias_t")
                nc.gpsimd.dma_start(out=bias_t[:], in_=bias_bcast_ap(s, half))
                n_mt = nb * MT
                for ml in range(n_mt):
                    b, mi = divmod(ml, MT)
                    last_tile = (s, half, ml) == (2, 1, n_mt - 1)
                    ps0 = ps_pool.tile([P, NH2], f32, tag="ps", name="ps0")
                    ps1 = ps_pool.tile([P, NH2], f32, tag="ps", name="ps1")
                    pss = (ps0, ps0, ps1, ps1)
                    for kt in range(KT):
                        lt = lhsT(bi0 + b, kt, mi)
                        for n4 in range(4):
                            nc.tensor.matmul(
                                pss[n4][:, (n4 % 2) * 512 : (n4 % 2 + 1) * 512],
                                lt,
                                w_t[:, kt, n4 * 512 : (n4 + 1) * 512],
                                start=(kt == 0),
                                stop=(kt == KT - 1),
                            )
                    dst = out_ap[
                        bi0 + b, mi * P : (mi + 1) * P, half * NH : (half + 1) * NH
                    ]
                    if last_tile:
                        # finer eviction pipeline to shorten the kernel tail
                        for nh in range(2):
                            out_t = outo_pool.tile([P, NH2], f16, tag="outo")
                            nc.vector.tensor_add(
                                out_t[:],
                                pss[2 * nh][:],
                                bias_t[:, nh * NH2 : (nh + 1) * NH2],
                            )
                            nc.scalar.dma_start(
                                dst[:, nh * NH2 : (nh + 1) * NH2], out_t[:]
                            )
                    else:
                        out_t = outm_pool.tile([P, NH], f16, tag="outm")
                        nc.vector.tensor_add(out_t[:, 0:NH2], ps0[:], bias_t[:, 0:NH2])
                        nc.vector.tensor_add(
                            out_t[:, NH2:NH], ps1[:], bias_t[:, NH2:NH]
                        )
                        nc.scalar.dma_start(dst, out_t[:])
    nc.compile()
    return nc


def _get_compiled():
    global _COMPILED
    if _COMPILED is None:
        _COMPILED = _build()
    return _COMPILED


def _pack(cat_ids):
    """Assign batches to cores with slot capacities [2,1,1] per core.

    Returns per-core (idx, slot_cats): idx = 4 batch indices ordered
    [pair0, pair1, single_b, single_c]; slot_cats = categories for the 3 slots.
    Always feasible: #disjoint same-cat pairs = (32 - #odd-count cats)/2 >= 8.
    """
    cat_ids = np.asarray(cat_ids)
    by_cat = {}
    for i, c in enumerate(cat_ids.tolist()):
        by_cat.setdefault(c, []).append(i)
    pairs = []
    singles = []
    for c, idxs in sorted(by_cat.items()):
        n = len(idxs)
        for j in range(n // 2):
            pairs.append((c, idxs[2 * j], idxs[2 * j + 1]))
        if n % 2:
            singles.append((c, idxs[-1]))
    assert len(pairs) >= N_CORES, "impossible: <8 same-cat pairs among 32 batches"
    core_pairs = pairs[:N_CORES]
    # leftovers: extra pairs flatten into singles
    for c, i, j in pairs[N_CORES:]:
        singles.append((c, i))
        singles.append((c, j))
    assert len(singles) == 2 * N_CORES
    cores = []
    for ci in range(N_CORES):
        c, i, j = core_pairs[ci]
        (cb, ib), (cc, ic) = singles[2 * ci], singles[2 * ci + 1]
        cores.append(([i, j, ib, ic], [c, cb, cc]))
    return cores


def run_sharded(x, cat_ids, W, b, trace=False, **spmd_kwargs):
    """Shard, run on 8 cores, unshard. Returns (out, BassKernelResults)."""
    x = np.ascontiguousarray(np.asarray(x), dtype=np.float32)
    cat_ids = np.asarray(cat_ids).astype(np.int64)
    W = np.ascontiguousarray(np.asarray(W), dtype=np.float32)
    b = np.ascontiguousarray(np.asarray(b), dtype=np.float32)

    nc = _get_compiled()
    cores = _pack(cat_ids)

    in_maps = []
    for idx, slot_cats in cores:
        # xt: [4, P, KT, S] with per-partition-contiguous [KT, S] lines.
        xt = np.ascontiguousarray(
            x[idx].astype(np.float16).reshape(BPC, S, KT, P).transpose(0, 3, 2, 1)
        )
        wsel = W[slot_cats].astype(np.float16)          # (3, K, H)
        # main W: [3, half, P, KT, 2048], 32 KB contiguous per partition.
        wh = np.ascontiguousarray(
            wsel.reshape(NSLOT, KT, P, NHALF, NH).transpose(0, 3, 2, 1, 4)
        )
        # opening W (slot A half 0): [nh, P, KT, 1024], 8 KB-contiguous kt runs.
        wo = np.ascontiguousarray(
            wsel[0, :, :NH].reshape(KT, P, 2, NH2).transpose(2, 1, 0, 3)
        )
        in_maps.append(
            {"xt": xt, "w": wh, "wo": wo, "bias": np.ascontiguousarray(b[slot_cats])}
        )

    res = run_bass_kernel_spmd(
        nc, in_maps, list(range(N_CORES)), trace=trace, **spmd_kwargs
    )

    out = np.empty((B, S, H), dtype=np.float32)
    for c, (idx, _) in enumerate(cores):
        out[idx] = res.results[c]["out"].astype(np.float32)
    return out, res


def kernel(x, cat_ids, W, b):
    out, _ = run_sharded(x, cat_ids, W, b)
    return out
